# revision 60
# baseline (speedup 1.0000x reference)
"""Trainium2 Bass kernel for nn_KeyRecorder (v7).

Math (reference):
  comp = LN(relu(obs @ W1 + b1)) * g1 + bl1          [B, T, R]
  past = max(comp[:, :-20:10, :], axis=time)          408 strided rows
  gmax = max(cummax(comp[:, -20:, :]), past)          [B, 20, R]
  out  = LN(relu(gmax @ W2 + b2)) * g2 + bl2          [B, 20, D]

Only 428 of the 4096 timesteps per batch element are consumed (408
strided + last 20); the host gathers those rows and ships them
transposed (d-major) in fp16.  Batch is sharded 2-per-core across 8
cores.

Structure (per core, 2 batch elems h=0,1; 428 tokens = 4 slabs of
107):
  - weights (W1 pack, W2 pack) ride the gpsimd DMA queue; obs pieces
    ride sync in need-order (obs0 2 halves, obs1 2 halves).  The DMA
    fabric gives a core ~165 GB/s aggregate no matter the queue
    count, so fewer/larger pieces win; the queue split just lets the
    first matmul (needs wp + obs0a) start ~1us earlier.
  - stage 1: W1-stationary matmuls -> psum z [64, 428]; DVE
    relu(z+b1) -> xr fp16 [r, t]; DVE x^2.
  - per-slab transpose matmul with [I64 | ones] lands [107 tok, 64
    feat + rowsum] in psum; per-slab 1-col matmul on x^2 gives ssq.
  - batched LN stats on [107, 4]: mu via ACT Identity(scale=1/R)
    (keeps the in-order DVE queue from blocking the other elem's
    relu), mu2/var on DVE, rstd via raw Rsqrt; apply (x-mu)*rstd all
    DVE; transpose back to [r, t] psum via id107 matmuls; p0 =
    reduce_max over slabs 0-2 emitted AFTER the applies (cumulative
    sem thresholds turn early emission into a false dependency);
    seeded tensor_tensor_scan over slab 3, cols 408..427 are gmax.
  - stage 3 joint across h: two [65,32]x[65,512] matmuls (tokens
    396..427; ones row adds b2') into one [64,512] psum at row
    offsets 0/32; ONE relu+rowsum, ONE square+ssq (DVE stt: the ACT
    accumulator read costs ~4x DVE's), ONE stats chain, ONE
    (x-mu)*rstd fp16 apply for both h; out rows 12:32 / 44:64 as two
    fp16 DMAs on the two queues; host upcasts.

Runtime notes: scalar-engine dma_start crashes the exec unit
(NRT_EXEC_UNIT_UNRECOVERABLE); stride-0 broadcast APs on DVE compile
but also crash the exec unit; gpsimd scalar_tensor_tensor fails the
walrus ISA check; tensor_tensor_reduce crashes; GPSIMD cannot read
PSUM (BIR verifier).  A dummy Rsqrt primes the one ACT table
(reciprocal_sqrt_and_small) serving Square/Identity/Rsqrt.  The
device throttles on utilization (a junk-matmul "p-state warmup" made
everything ~20% slower), so total engine activity is kept minimal.
Engine queues execute in order, so op placement/emission order
controls head-of-line blocking; cross-engine dep latency is
~100-200ns per hop.

Affine folds (host side): LN1's g1/bl1 fold into W2/b2 (g1 >= 0
asserted; max/cummax commute with monotone maps); LN2's g2/bl2 are
applied to the gathered output on the host.
"""

import os
import numpy as np

import concourse.bass as bass
import concourse.bacc as bacc
import concourse.mybir as mybir
import concourse.tile as tile
from concourse.bass_utils import run_bass_kernel_spmd

F32 = mybir.dt.float32
F16 = mybir.dt.float16
ALU = mybir.AluOpType
ACT = mybir.ActivationFunctionType

B, T, D, R = 16, 4096, 512, 64
LOCAL, SR, EPS = 20, 10, 1e-5
N_CORES = 8
BPC = B // N_CORES                   # batch elements per core
NSTR = (T - LOCAL + SR - 1) // SR    # 408 strided past rows
NSEL = NSTR + LOCAL                  # 428 rows consumed per batch elem
GRP = NSEL                           # no padding
SLAB = 107                           # 4 slabs per batch element
NSLB = 4
DC = D // 128                        # 4 contraction chunks
NO = BPC * LOCAL                     # 40 output rows per core
S3W = 32                             # stage-3 stationary width (>=20,
                                     # mult of 32 for tile_position)
S3LO = GRP - S3W                     # first token col in stage-3 (396)

# packed weight tensor column offsets (fp16)
WCOL_IDP = 256                       # [I64 | ones] for the transpose
WCOL_ID107 = WCOL_IDP + R + 1        # 321: id107 for transpose-back
WCOL_B1 = WCOL_ID107 + SLAB          # 428
WP1 = WCOL_B1 + 1                    # 429 cols

IDX = np.array(list(range(0, T - LOCAL, SR)) + list(range(T - LOCAL, T)))

_cache: dict = {}


def _build_program():
    if "nc" in _cache:
        return _cache["nc"]

    nc = bacc.Bacc("TRN2", target_bir_lowering=False, debug=False,
                   enable_asserts=False)

    obs_d = [nc.dram_tensor(f"obs{h}", [128, DC, GRP], F16,
                            kind="ExternalInput") for h in range(BPC)]
    wp_d = nc.dram_tensor("wpack", [128, WP1], F16, kind="ExternalInput")
    w2_d = nc.dram_tensor("w2pack", [R + 1, D], F16, kind="ExternalInput")
    out_d = nc.dram_tensor("out", [NO, D], F16, kind="ExternalOutput")

    inv_r = 1.0 / R
    inv_d = 1.0 / D

    with tile.TileContext(nc) as tc:
        with (
            tc.tile_pool(name="const", bufs=1) as cpool,
            tc.tile_pool(name="pg", bufs=2, space=bass.MemorySpace.PSUM) as ppg,
            tc.tile_pool(name="xr", bufs=2, space=bass.MemorySpace.PSUM) as pxr,
            tc.tile_pool(name="ct", bufs=2, space=bass.MemorySpace.PSUM) as pct,
            tc.tile_pool(name="o3", bufs=1, space=bass.MemorySpace.PSUM) as po3,
        ):
            # ---------- SBUF tiles ----------
            wp = cpool.tile([128, WP1], F16)
            w2 = cpool.tile([R + 1, D], F16)
            dmy = cpool.tile([1, 1], F32)
            dmyo = cpool.tile([1, 1], F32)
            neginf = cpool.tile([R, GRP], F16)
            eps_t = cpool.tile([128, 1], F32)
            b1f = cpool.tile([R, 1], F32)
            obs_sb, st, scr = [], [], []
            for h in range(BPC):
                obs_sb.append(cpool.tile([128, DC, GRP], F16,
                                         name=f"obs_sb{h}"))
                scr.append(cpool.tile([R + 1, GRP], F16, name=f"scr{h}"))
                sh = dict(xr=([R, GRP], F16),
                          sq=([R, GRP], F16),
                          y=([SLAB, NSLB, R], F16),
                          p0=([R, 1], F32),
                          mu=([SLAB, NSLB], F32),
                          mu2=([SLAB, NSLB], F32),
                          var=([SLAB, NSLB], F32),
                          rstd=([SLAB, NSLB], F32),
                          nmr=([SLAB, NSLB], F32))
                st.append({k: cpool.tile(s, dt, name=f"s1_{k}{h}")
                           for k, (s, dt) in sh.items()})
            sh3 = dict(xr=([R, D], F16), sq=([R, D], F16),
                       rsum=([R, 1], F32), ssq=([R, 1], F32),
                       mu=([R, 1], F32), mu2=([R, 1], F32),
                       var=([R, 1], F32), rstd=([R, 1], F32),
                       out=([R, D], F16))
            u = {k: cpool.tile(s, dt, name=f"s3_{k}")
                 for k, (s, dt) in sh3.items()}

            w1c = lambda c: wp[:, 64 * c:64 * (c + 1)]
            idp = wp[0:R, WCOL_IDP:WCOL_IDP + R + 1]
            id107 = wp[0:SLAB, WCOL_ID107:WCOL_ID107 + SLAB]
            b1col = wp[0:R, WCOL_B1:WCOL_B1 + 1]

            # ---------- DMA in ----------
            # The DMA fabric gives this core ~160 GB/s aggregate no
            # matter how many queues are used, so inputs ride ONE sync
            # queue in need-order (the sync engine also issues ~0.7us
            # earlier than gpsimd); only w2 (needed last) rides gpsimd.
            # weights ride gpsimd so the sync queue's first piece is
            # obs0a (the first matmul needs wp AND obs0a; both land
            # ~0.9us earlier this way); obs pieces in need-order on
            # sync, ~165 GB/s aggregate regardless of queue count.
            nc.gpsimd.dma_start(wp[:], wp_d[:])
            nc.gpsimd.dma_start(w2[:], w2_d[:])
            nc.sync.dma_start(obs_sb[0][:, 0:2, :], obs_d[0][:, 0:2, :])
            nc.sync.dma_start(obs_sb[0][:, 2:4, :], obs_d[0][:, 2:4, :])
            nc.sync.dma_start(obs_sb[1][:, 0:2, :], obs_d[1][:, 0:2, :])
            nc.sync.dma_start(obs_sb[1][:, 2:4, :], obs_d[1][:, 2:4, :])

            # ---------- constants (vector; off the DMA queues) ----------
            nc.vector.memset(dmy[:], 1.0)
            nc.vector.memset(eps_t[:], EPS)
            nc.vector.memset(neginf[:, 0:SLAB], -60000.0)
            nc.vector.memset(scr[0][R:R + 1, S3LO:GRP], 1.0)
            nc.vector.memset(scr[1][R:R + 1, S3LO:GRP], 1.0)

            # raw Rsqrt activation: the bass wrapper refuses Rsqrt on
            # accuracy grounds; ~1e-3 relative is fine here and it fuses
            # sqrt+reciprocal into one op.
            def rsqrt_act(out, in_, bias):
                eng = nc.scalar
                ins_ = [eng.lower_ap(in_), eng.lower_ap(bias),
                        mybir.ImmediateValue(dtype=mybir.dt.float32,
                                             value=1.0),
                        mybir.ImmediateValue(dtype=mybir.dt.float32,
                                             value=0.0)]
                return eng.add_instruction(mybir.InstActivation(
                    name=eng.bass.get_next_instruction_name(),
                    func=ACT.Rsqrt, ins=ins_, outs=[eng.lower_ap(out)]))

            # primes the single ACT table (reciprocal_sqrt_and_small)
            rsqrt_act(dmyo[:], dmy[:], eps_t[0:1, :])

            # b1 as fp32 (DVE tensor_scalar add needs an fp32 scalar AP)
            nc.gpsimd.tensor_scalar_mul(b1f[:], b1col, 1.0)

            # NOTE: a PE "p-state warmup" (junk matmuls during the DMA
            # wait) was tried and made EVERYTHING ~20% slower — the
            # device applies a utilization-based power throttle, so
            # total engine activity must be kept minimal.

            def pipeline(h):
                t = st[h]
                obs = obs_sb[h]

                # stage 1: z = obs @ W1 into psum [64, 428]
                pg = ppg.tile([R, GRP], F32, tag="pg")
                for c in range(DC):
                    nc.tensor.matmul(pg[:], w1c(c), obs[:, c, :],
                                     start=(c == 0), stop=(c == DC - 1))
                # relu(z + b1) -> xr fp16 [r, t] (DVE); x^2 via DVE
                # fp16 self-multiply (feeds the ssq colsum matmuls)
                nc.vector.tensor_scalar(t["xr"][:], pg[:], b1f[:], 0.0,
                                        ALU.add, ALU.max)
                nc.vector.tensor_tensor(t["sq"][:], t["xr"][:],
                                        t["xr"][:], op=ALU.mult)

                # per-slab transpose: cols 0..63 = x, col 64 = rowsum
                # (ones col of idp), col 65 = sum of squares.  mu/mu2
                # are emitted BETWEEN the two matmul loops: semaphore
                # thresholds follow emission order, so this keeps mu
                # from falsely waiting on the sq matmuls.  mu rides ACT
                # (a DVE mu would block the other elem's ready relu in
                # the in-order DVE queue).
                xrp = pxr.tile([SLAB, NSLB, R + 2], F32, tag="xr")
                for j in range(NSLB):
                    nc.tensor.matmul(xrp[:, j, 0:R + 1],
                                     t["xr"][:, SLAB * j:SLAB * (j + 1)],
                                     idp, start=True, stop=True)
                nc.scalar.activation(t["mu"][:], xrp[:, :, R],
                                     ACT.Identity, scale=inv_r)
                nc.vector.tensor_tensor(t["mu2"][:], t["mu"][:],
                                        t["mu"][:], op=ALU.mult)
                for j in range(NSLB):
                    nc.tensor.matmul(xrp[:, j, R + 1:R + 2],
                                     t["sq"][:, SLAB * j:SLAB * (j + 1)],
                                     idp[:, R:R + 1], start=True,
                                     stop=True)

                # batched LN stats on [107, 4]
                nc.vector.scalar_tensor_tensor(t["var"][:],
                                               xrp[:, :, R + 1],
                                               inv_r, t["mu2"][:],
                                               ALU.mult, ALU.subtract)
                rsqrt_act(t["rstd"][:], t["var"][:], eps_t[0:SLAB, :])

                # apply LN + transpose back to [r, t] psum.  h1 (the
                # critical tail) gets the DVE to itself for its
                # applies + both p0/scans; h0's applies ride ACT via
                # nmr = -mu*rstd (the gpsimd nmr hop also delays them
                # past rstd_h1's ACT slot).  A broadcast-AP batched
                # variant compiles but crashes the exec unit.
                nc.gpsimd.tensor_scalar_mul(t["nmr"][:], t["mu"][:],
                                            -1.0)
                nc.gpsimd.tensor_tensor(t["nmr"][:], t["nmr"][:],
                                        t["rstd"][:], op=ALU.mult)
                ct = pct.tile([R, GRP], F32, tag="ct")
                for j in range(NSLB):
                    if h == 0 or j == 0:
                        # h0 fully on ACT; h1's slab 0 also rides ACT
                        # (free after h0's applies), shortening the
                        # DVE-serial tail by one apply
                        nc.scalar.activation(t["y"][:, j, :],
                                             xrp[:, j, 0:R],
                                             ACT.Identity,
                                             bias=t["nmr"][:, j:j + 1],
                                             scale=t["rstd"][:, j:j + 1])
                    else:
                        nc.vector.tensor_scalar(t["y"][:, j, :],
                                                xrp[:, j, 0:R],
                                                t["mu"][:, j:j + 1],
                                                t["rstd"][:, j:j + 1],
                                                ALU.subtract, ALU.mult)
                    nc.tensor.matmul(ct[:, SLAB * j:SLAB * (j + 1)],
                                     t["y"][:, j, :], id107,
                                     start=True, stop=True)

                # p0 emitted AFTER the applies: DVE sem thresholds are
                # cumulative, so an earlier emission makes backMM3 wait
                # on p0 (false dependency)
                nc.vector.reduce_max(t["p0"][:], ct[:, 0:3 * SLAB],
                                     axis=mybir.AxisListType.X)

                # seeded running max through the last slab's cols;
                # cols 408..427 (of 321..428) are gmax
                nc.vector.tensor_tensor_scan(
                    scr[h][0:R, 3 * SLAB:GRP], ct[:, 3 * SLAB:GRP],
                    neginf[:, 0:SLAB], t["p0"][:], ALU.max, ALU.max)

            pipeline(0)
            pipeline(1)

            # ---------- stage 3, joint across h ----------
            ps3 = po3.tile([2 * S3W, D], F32, tag="o3")
            for h in range(BPC):
                nc.tensor.matmul(ps3[S3W * h:S3W * (h + 1), :],
                                 scr[h][:, S3LO:GRP], w2[:],
                                 start=True, stop=True)
            # relu + rowsum, then square + ssq (both DVE; the ACT
            # accumulator read is ~4x costlier than DVE's)
            nc.vector.tensor_scalar(u["xr"][:], ps3[:], 0.0, 0.0,
                                    ALU.max, ALU.add,
                                    accum_out=u["rsum"][:])
            nc.vector.scalar_tensor_tensor(u["sq"][:], u["xr"][:], 1.0,
                                           u["xr"][:], ALU.mult,
                                           ALU.mult,
                                           accum_out=u["ssq"][:])
            # mu on ACT concurrent with sq3 on DVE (gpsimd is far
            # slower and triggers a mid-kernel library reload)
            nc.scalar.activation(u["mu"][:], u["rsum"][:],
                                 ACT.Identity, scale=inv_d)
            nc.vector.tensor_tensor(u["mu2"][:], u["mu"][:], u["mu"][:],
                                    op=ALU.mult)
            nc.vector.scalar_tensor_tensor(u["var"][:], u["ssq"][:],
                                           inv_d, u["mu2"][:],
                                           ALU.mult, ALU.subtract)
            rsqrt_act(u["rstd"][:], u["var"][:], eps_t[0:R, :])
            nc.vector.tensor_scalar(u["out"][:], u["xr"][:], u["mu"][:],
                                    u["rstd"][:], ALU.subtract, ALU.mult)
            # useful rows: 12:32 (h0 tokens 408..427), 44:64 (h1);
            # two parallel fp16 DMAs on the two idle queues; host
            # upcasts
            nc.gpsimd.dma_start(out_d[0:LOCAL, :],
                                u["out"][S3W - LOCAL:S3W, :])
            nc.sync.dma_start(out_d[LOCAL:NO, :],
                              u["out"][2 * S3W - LOCAL:2 * S3W, :])

    nc.compile()
    _cache["nc"] = nc
    return nc


def _host_inputs(obs, W1, b1, ln1_g, ln1_b, W2, b2):
    obs = np.ascontiguousarray(np.asarray(obs, dtype=np.float32))
    W1 = np.asarray(W1, np.float32)
    b1 = np.asarray(b1, np.float32)
    ln1_g = np.asarray(ln1_g, np.float32)
    ln1_b = np.asarray(ln1_b, np.float32)
    W2 = np.asarray(W2, np.float32)
    b2 = np.asarray(b2, np.float32)

    # folding LN1's affine past the max/cummax requires monotonicity
    assert np.all(ln1_g >= 0), "ln1_g must be >= 0 for the affine fold"

    wpack = np.zeros((128, WP1), np.float16)
    wpack[:, 0:256] = W1.reshape(DC, 128, R).transpose(1, 0, 2).reshape(
        128, 256).astype(np.float16)
    wpack[0:R, WCOL_IDP:WCOL_IDP + R] = np.eye(R, dtype=np.float16)
    wpack[0:R, WCOL_IDP + R] = 1.0
    wpack[0:SLAB, WCOL_ID107:WCOL_ID107 + SLAB] = np.eye(
        SLAB, dtype=np.float16)
    wpack[0:R, WCOL_B1] = b1.astype(np.float16)

    w2pack = np.concatenate(
        [ln1_g[:, None] * W2, (b2 + ln1_b @ W2)[None, :]],
        axis=0).astype(np.float16)

    shared = {"wpack": wpack, "w2pack": np.ascontiguousarray(w2pack)}
    in_maps = []
    for c in range(N_CORES):
        sel = obs[BPC * c:BPC * (c + 1)][:, IDX, :]        # [BPC, 428, 512]
        m = {}
        for h in range(BPC):
            obsT = sel[h].T                                 # [512, 428]
            obsf = obsT.reshape(DC, 128, GRP).transpose(1, 0, 2)
            m[f"obs{h}"] = np.ascontiguousarray(obsf.astype(np.float16))
        in_maps.append({**m, **shared})
    return in_maps


def _install_ntff_shim():
    """The agent image's antenv lacks axon_hooks; synthesize it so
    trace=True can reach the libaxon NTFF profiler (test-time only)."""
    import sys
    import types
    if "antenv.axon_hooks" in sys.modules:
        return True
    try:
        import antenv
        from trn_agent_boot.trn_boot import _ntff_profile_via_ctypes
    except ImportError:
        return False
    so_path = "/opt/axon/libaxon_pjrt.so"
    if not os.path.exists(so_path):
        return False
    hook = _ntff_profile_via_ctypes(so_path)
    mod = types.ModuleType("antenv.axon_hooks")
    mod._hook = hook
    mod.set_axon_ntff_profile_hook = lambda h: setattr(mod, "_hook", h)
    mod.get_axon_ntff_profile_hook = lambda: mod._hook
    sys.modules["antenv.axon_hooks"] = mod
    antenv.axon_hooks = mod
    return hook is not None


def kernel(obs_frames, W1, b1, ln1_g, ln1_b, W2, b2, ln2_g, ln2_b):
    nc = _build_program()
    in_maps = _host_inputs(obs_frames, W1, b1, ln1_g, ln1_b, W2, b2)
    trace = bool(os.environ.get("BASS_TRACE"))
    if trace:
        trace = _install_ntff_shim()
        import concourse.bass_utils as _bu
        _bu.upload_artifacts = lambda tmpdir: f"local://{tmpdir}"
    res = run_bass_kernel_spmd(nc, in_maps, core_ids=list(range(N_CORES)),
                               trace=trace)
    _cache["last_result"] = res
    out = np.stack([res.results[c]["out"].astype(np.float32)
                    .reshape(BPC, LOCAL, D) for c in range(N_CORES)])
    out = out.reshape(B, LOCAL, D)

    # LN2's affine applied host-side (identity for the given inputs)
    g2 = np.asarray(ln2_g, np.float32)
    b2l = np.asarray(ln2_b, np.float32)
    if not (np.all(g2 == 1.0) and np.all(b2l == 0.0)):
        out = out * g2 + b2l
    return np.ascontiguousarray(out.astype(np.float32))


# revision 61
# speedup vs baseline: 1.0917x; 1.0917x over previous
"""Trainium2 Bass kernel for nn_KeyRecorder (v7).

Math (reference):
  comp = LN(relu(obs @ W1 + b1)) * g1 + bl1          [B, T, R]
  past = max(comp[:, :-20:10, :], axis=time)          408 strided rows
  gmax = max(cummax(comp[:, -20:, :]), past)          [B, 20, R]
  out  = LN(relu(gmax @ W2 + b2)) * g2 + bl2          [B, 20, D]

Only 428 of the 4096 timesteps per batch element are consumed (408
strided + last 20); the host gathers those rows and ships them
transposed (d-major) in fp16.  Batch is sharded 2-per-core across 8
cores.

Structure (per core, 2 batch elems h=0,1; 428 tokens = 4 slabs of
107):
  - weights (W1 pack, W2 pack) ride the gpsimd DMA queue; obs pieces
    ride sync in need-order (obs0 2 halves, obs1 2 halves).  The DMA
    fabric gives a core ~165 GB/s aggregate no matter the queue
    count, so fewer/larger pieces win; the queue split just lets the
    first matmul (needs wp + obs0a) start ~1us earlier.
  - stage 1: W1-stationary matmuls -> psum z [64, 428]; DVE
    relu(z+b1) -> xr fp16 [r, t]; DVE x^2.
  - per-slab transpose matmul with [I64 | ones] lands [107 tok, 64
    feat + rowsum] in psum; per-slab 1-col matmul on x^2 gives ssq.
  - batched LN stats on [107, 4]: mu via ACT Identity(scale=1/R)
    (keeps the in-order DVE queue from blocking the other elem's
    relu), mu2/var on DVE, rstd via raw Rsqrt; apply (x-mu)*rstd all
    DVE; transpose back to [r, t] psum via id107 matmuls; p0 =
    reduce_max over slabs 0-2 emitted AFTER the applies (cumulative
    sem thresholds turn early emission into a false dependency);
    seeded tensor_tensor_scan over slab 3, cols 408..427 are gmax.
  - stage 3 joint across h: two [65,32]x[65,512] matmuls (tokens
    396..427; ones row adds b2') into one [64,512] psum at row
    offsets 0/32; ONE relu+rowsum, ONE square+ssq (DVE stt: the ACT
    accumulator read costs ~4x DVE's), ONE stats chain, ONE
    (x-mu)*rstd fp16 apply for both h; out rows 12:32 / 44:64 as two
    fp16 DMAs on the two queues; host upcasts.

Runtime notes: scalar-engine dma_start crashes the exec unit
(NRT_EXEC_UNIT_UNRECOVERABLE); stride-0 broadcast APs on DVE compile
but also crash the exec unit; gpsimd scalar_tensor_tensor fails the
walrus ISA check; tensor_tensor_reduce crashes; GPSIMD cannot read
PSUM (BIR verifier).  A dummy Rsqrt primes the one ACT table
(reciprocal_sqrt_and_small) serving Square/Identity/Rsqrt.  The
device throttles on utilization (a junk-matmul "p-state warmup" made
everything ~20% slower), so total engine activity is kept minimal.
Engine queues execute in order, so op placement/emission order
controls head-of-line blocking; cross-engine dep latency is
~100-200ns per hop.

Affine folds (host side): LN1's g1/bl1 fold into W2/b2 (g1 >= 0
asserted; max/cummax commute with monotone maps); LN2's g2/bl2 are
applied to the gathered output on the host.
"""

import os
import numpy as np

import concourse.bass as bass
import concourse.bacc as bacc
import concourse.mybir as mybir
import concourse.tile as tile
from concourse.bass_utils import run_bass_kernel_spmd

F32 = mybir.dt.float32
F16 = mybir.dt.float16
ALU = mybir.AluOpType
ACT = mybir.ActivationFunctionType

B, T, D, R = 16, 4096, 512, 64
LOCAL, SR, EPS = 20, 10, 1e-5
N_CORES = 8
BPC = B // N_CORES                   # batch elements per core
NSTR = (T - LOCAL + SR - 1) // SR    # 408 strided past rows
NSEL = NSTR + LOCAL                  # 428 rows consumed per batch elem
GRP = NSEL                           # no padding
SLAB = 107                           # 4 slabs per batch element
NSLB = 4
DC = D // 128                        # 4 contraction chunks
NO = BPC * LOCAL                     # 40 output rows per core
S3W = 32                             # stage-3 stationary width (>=20,
                                     # mult of 32 for tile_position)
S3LO = GRP - S3W                     # first token col in stage-3 (396)

# packed weight tensor column offsets (fp16)
WCOL_IDP = 256                       # [I64 | ones] for the transpose
WCOL_ID107 = WCOL_IDP + R + 1        # 321: id107 for transpose-back
WCOL_B1 = WCOL_ID107 + SLAB          # 428
WP1 = WCOL_B1 + 1                    # 429 cols

IDX = np.array(list(range(0, T - LOCAL, SR)) + list(range(T - LOCAL, T)))

_cache: dict = {}


def _build_program():
    if "nc" in _cache:
        return _cache["nc"]

    nc = bacc.Bacc("TRN2", target_bir_lowering=False, debug=False,
                   enable_asserts=False)

    obs_d = [nc.dram_tensor(f"obs{h}", [128, DC, GRP], F16,
                            kind="ExternalInput") for h in range(BPC)]
    wp_d = nc.dram_tensor("wpack", [128, WP1], F16, kind="ExternalInput")
    w2_d = nc.dram_tensor("w2pack", [R + 1, D], F16, kind="ExternalInput")
    out_d = nc.dram_tensor("out", [NO, D], F16, kind="ExternalOutput")

    inv_r = 1.0 / R
    inv_d = 1.0 / D

    with tile.TileContext(nc) as tc:
        with (
            tc.tile_pool(name="const", bufs=1) as cpool,
            tc.tile_pool(name="pg", bufs=2, space=bass.MemorySpace.PSUM) as ppg,
            tc.tile_pool(name="xr", bufs=2, space=bass.MemorySpace.PSUM) as pxr,
            tc.tile_pool(name="ct", bufs=2, space=bass.MemorySpace.PSUM) as pct,
            tc.tile_pool(name="o3", bufs=1, space=bass.MemorySpace.PSUM) as po3,
        ):
            # ---------- SBUF tiles ----------
            wp = cpool.tile([128, WP1], F16)
            w2 = cpool.tile([R + 1, D], F16)
            dmy = cpool.tile([1, 1], F32)
            dmyo = cpool.tile([1, 1], F32)
            neginf = cpool.tile([R, GRP], F16)
            eps_t = cpool.tile([128, 1], F32)
            b1f = cpool.tile([R, 1], F32)
            obs_sb, st, scr = [], [], []
            for h in range(BPC):
                obs_sb.append(cpool.tile([128, DC, GRP], F16,
                                         name=f"obs_sb{h}"))
                scr.append(cpool.tile([R + 1, GRP], F16, name=f"scr{h}"))
                sh = dict(xr=([R, GRP], F16),
                          sq=([R, GRP], F16),
                          y=([SLAB, NSLB, R], F16),
                          p0=([R, 1], F32),
                          mu=([SLAB, NSLB], F32),
                          mu2=([SLAB, NSLB], F32),
                          var=([SLAB, NSLB], F32),
                          rstd=([SLAB, NSLB], F32),
                          nmr=([SLAB, NSLB], F32))
                st.append({k: cpool.tile(s, dt, name=f"s1_{k}{h}")
                           for k, (s, dt) in sh.items()})
            sh3 = dict(xr=([R, D], F16), sq=([R, D], F16),
                       rsum=([R, 1], F32), ssq=([R, 1], F32),
                       mu=([R, 1], F32), mu2=([R, 1], F32),
                       var=([R, 1], F32), rstd=([R, 1], F32),
                       out=([R, D], F16))
            u = {k: cpool.tile(s, dt, name=f"s3_{k}")
                 for k, (s, dt) in sh3.items()}

            w1c = lambda c: wp[:, 64 * c:64 * (c + 1)]
            idp = wp[0:R, WCOL_IDP:WCOL_IDP + R + 1]
            id107 = wp[0:SLAB, WCOL_ID107:WCOL_ID107 + SLAB]
            b1col = wp[0:R, WCOL_B1:WCOL_B1 + 1]

            # ---------- DMA in ----------
            # The DMA fabric gives this core ~160 GB/s aggregate no
            # matter how many queues are used, so inputs ride ONE sync
            # queue in need-order (the sync engine also issues ~0.7us
            # earlier than gpsimd); only w2 (needed last) rides gpsimd.
            # weights ride gpsimd so the sync queue's first piece is
            # obs0a (the first matmul needs wp AND obs0a; both land
            # ~0.9us earlier this way); obs pieces in need-order on
            # sync, ~165 GB/s aggregate regardless of queue count.
            nc.gpsimd.dma_start(wp[:], wp_d[:])
            nc.gpsimd.dma_start(w2[:], w2_d[:])
            nc.sync.dma_start(obs_sb[0][:, 0:2, :], obs_d[0][:, 0:2, :])
            nc.sync.dma_start(obs_sb[0][:, 2:4, :], obs_d[0][:, 2:4, :])
            nc.sync.dma_start(obs_sb[1][:, 0:2, :], obs_d[1][:, 0:2, :])
            nc.sync.dma_start(obs_sb[1][:, 2:4, :], obs_d[1][:, 2:4, :])

            # ---------- constants (vector; off the DMA queues) ----------
            nc.vector.memset(dmy[:], 1.0)
            nc.vector.memset(eps_t[:], EPS)
            nc.vector.memset(neginf[:, 0:SLAB], -60000.0)
            nc.vector.memset(scr[0][R:R + 1, S3LO:GRP], 1.0)
            nc.vector.memset(scr[1][R:R + 1, S3LO:GRP], 1.0)

            # raw Rsqrt activation: the bass wrapper refuses Rsqrt on
            # accuracy grounds; ~1e-3 relative is fine here and it fuses
            # sqrt+reciprocal into one op.
            def rsqrt_act(out, in_, bias):
                eng = nc.scalar
                ins_ = [eng.lower_ap(in_), eng.lower_ap(bias),
                        mybir.ImmediateValue(dtype=mybir.dt.float32,
                                             value=1.0),
                        mybir.ImmediateValue(dtype=mybir.dt.float32,
                                             value=0.0)]
                return eng.add_instruction(mybir.InstActivation(
                    name=eng.bass.get_next_instruction_name(),
                    func=ACT.Rsqrt, ins=ins_, outs=[eng.lower_ap(out)]))

            # primes the single ACT table (reciprocal_sqrt_and_small)
            rsqrt_act(dmyo[:], dmy[:], eps_t[0:1, :])

            # b1 as fp32 (DVE tensor_scalar add needs an fp32 scalar AP)
            nc.gpsimd.tensor_scalar_mul(b1f[:], b1col, 1.0)

            # NOTE: a PE "p-state warmup" (junk matmuls during the DMA
            # wait) was tried and made EVERYTHING ~20% slower — the
            # device applies a utilization-based power throttle, so
            # total engine activity must be kept minimal.

            def pipeline(h):
                t = st[h]
                obs = obs_sb[h]

                # stage 1: z = obs @ W1 into psum [64, 428]
                pg = ppg.tile([R, GRP], F32, tag="pg")
                for c in range(DC):
                    nc.tensor.matmul(pg[:], w1c(c), obs[:, c, :],
                                     start=(c == 0), stop=(c == DC - 1))
                # relu(z + b1) -> xr fp16 [r, t] (DVE); x^2 via DVE
                # fp16 self-multiply (feeds the ssq colsum matmuls)
                nc.vector.tensor_scalar(t["xr"][:], pg[:], b1f[:], 0.0,
                                        ALU.add, ALU.max)
                nc.vector.tensor_tensor(t["sq"][:], t["xr"][:],
                                        t["xr"][:], op=ALU.mult)

                # per-slab transpose: cols 0..63 = x, col 64 = rowsum
                # (ones col of idp), col 65 = sum of squares.  mu/mu2
                # are emitted BETWEEN the two matmul loops: semaphore
                # thresholds follow emission order, so this keeps mu
                # from falsely waiting on the sq matmuls.  mu rides ACT
                # (a DVE mu would block the other elem's ready relu in
                # the in-order DVE queue).
                xrp = pxr.tile([SLAB, NSLB, R + 2], F32, tag="xr")
                for j in range(NSLB):
                    nc.tensor.matmul(xrp[:, j, 0:R + 1],
                                     t["xr"][:, SLAB * j:SLAB * (j + 1)],
                                     idp, start=True, stop=True)
                nc.scalar.activation(t["mu"][:], xrp[:, :, R],
                                     ACT.Identity, scale=inv_r)
                nc.vector.tensor_tensor(t["mu2"][:], t["mu"][:],
                                        t["mu"][:], op=ALU.mult)
                for j in range(NSLB):
                    nc.tensor.matmul(xrp[:, j, R + 1:R + 2],
                                     t["sq"][:, SLAB * j:SLAB * (j + 1)],
                                     idp[:, R:R + 1], start=True,
                                     stop=True)

                # batched LN stats on [107, 4]
                nc.vector.scalar_tensor_tensor(t["var"][:],
                                               xrp[:, :, R + 1],
                                               inv_r, t["mu2"][:],
                                               ALU.mult, ALU.subtract)
                rsqrt_act(t["rstd"][:], t["var"][:], eps_t[0:SLAB, :])

                # apply LN + transpose back to [r, t] psum.  h1 (the
                # critical tail) gets the DVE to itself for its
                # applies + both p0/scans; h0's applies ride ACT via
                # nmr = -mu*rstd (the gpsimd nmr hop also delays them
                # past rstd_h1's ACT slot).  A broadcast-AP batched
                # variant compiles but crashes the exec unit.
                nc.gpsimd.tensor_scalar_mul(t["nmr"][:], t["mu"][:],
                                            -1.0)
                nc.gpsimd.tensor_tensor(t["nmr"][:], t["nmr"][:],
                                        t["rstd"][:], op=ALU.mult)
                ct = pct.tile([R, GRP], F32, tag="ct")
                for j in range(NSLB):
                    if h == 0 or j == 3:
                        # h0 fully on ACT; h1's slab 3 also rides ACT
                        # (free after h0's applies): p0 only needs
                        # slabs 0-2, so slab 3's apply overlaps p0 and
                        # the DVE-serial tail drops by one apply
                        nc.scalar.activation(t["y"][:, j, :],
                                             xrp[:, j, 0:R],
                                             ACT.Identity,
                                             bias=t["nmr"][:, j:j + 1],
                                             scale=t["rstd"][:, j:j + 1])
                    else:
                        nc.vector.tensor_scalar(t["y"][:, j, :],
                                                xrp[:, j, 0:R],
                                                t["mu"][:, j:j + 1],
                                                t["rstd"][:, j:j + 1],
                                                ALU.subtract, ALU.mult)
                    nc.tensor.matmul(ct[:, SLAB * j:SLAB * (j + 1)],
                                     t["y"][:, j, :], id107,
                                     start=True, stop=True)

                # p0 emitted AFTER the applies: DVE sem thresholds are
                # cumulative, so an earlier emission makes backMM3 wait
                # on p0 (false dependency)
                nc.vector.reduce_max(t["p0"][:], ct[:, 0:3 * SLAB],
                                     axis=mybir.AxisListType.X)

                # seeded running max through the last slab's cols;
                # cols 408..427 (of 321..428) are gmax
                nc.vector.tensor_tensor_scan(
                    scr[h][0:R, 3 * SLAB:GRP], ct[:, 3 * SLAB:GRP],
                    neginf[:, 0:SLAB], t["p0"][:], ALU.max, ALU.max)

            pipeline(0)
            pipeline(1)

            # ---------- stage 3, joint across h ----------
            ps3 = po3.tile([2 * S3W, D], F32, tag="o3")
            for h in range(BPC):
                nc.tensor.matmul(ps3[S3W * h:S3W * (h + 1), :],
                                 scr[h][:, S3LO:GRP], w2[:],
                                 start=True, stop=True)
            # relu + rowsum, then square + ssq (both DVE; the ACT
            # accumulator read is ~4x costlier than DVE's)
            nc.vector.tensor_scalar(u["xr"][:], ps3[:], 0.0, 0.0,
                                    ALU.max, ALU.add,
                                    accum_out=u["rsum"][:])
            nc.vector.scalar_tensor_tensor(u["sq"][:], u["xr"][:], 1.0,
                                           u["xr"][:], ALU.mult,
                                           ALU.mult,
                                           accum_out=u["ssq"][:])
            # mu on ACT concurrent with sq3 on DVE (gpsimd is far
            # slower and triggers a mid-kernel library reload)
            nc.scalar.activation(u["mu"][:], u["rsum"][:],
                                 ACT.Identity, scale=inv_d)
            nc.vector.tensor_tensor(u["mu2"][:], u["mu"][:], u["mu"][:],
                                    op=ALU.mult)
            nc.vector.scalar_tensor_tensor(u["var"][:], u["ssq"][:],
                                           inv_d, u["mu2"][:],
                                           ALU.mult, ALU.subtract)
            rsqrt_act(u["rstd"][:], u["var"][:], eps_t[0:R, :])
            nc.vector.tensor_scalar(u["out"][:], u["xr"][:], u["mu"][:],
                                    u["rstd"][:], ALU.subtract, ALU.mult)
            # useful rows: 12:32 (h0 tokens 408..427), 44:64 (h1);
            # two parallel fp16 DMAs on the two idle queues; host
            # upcasts
            nc.gpsimd.dma_start(out_d[0:LOCAL, :],
                                u["out"][S3W - LOCAL:S3W, :])
            nc.sync.dma_start(out_d[LOCAL:NO, :],
                              u["out"][2 * S3W - LOCAL:2 * S3W, :])

    nc.compile()
    _cache["nc"] = nc
    return nc


def _host_inputs(obs, W1, b1, ln1_g, ln1_b, W2, b2):
    obs = np.ascontiguousarray(np.asarray(obs, dtype=np.float32))
    W1 = np.asarray(W1, np.float32)
    b1 = np.asarray(b1, np.float32)
    ln1_g = np.asarray(ln1_g, np.float32)
    ln1_b = np.asarray(ln1_b, np.float32)
    W2 = np.asarray(W2, np.float32)
    b2 = np.asarray(b2, np.float32)

    # folding LN1's affine past the max/cummax requires monotonicity
    assert np.all(ln1_g >= 0), "ln1_g must be >= 0 for the affine fold"

    wpack = np.zeros((128, WP1), np.float16)
    wpack[:, 0:256] = W1.reshape(DC, 128, R).transpose(1, 0, 2).reshape(
        128, 256).astype(np.float16)
    wpack[0:R, WCOL_IDP:WCOL_IDP + R] = np.eye(R, dtype=np.float16)
    wpack[0:R, WCOL_IDP + R] = 1.0
    wpack[0:SLAB, WCOL_ID107:WCOL_ID107 + SLAB] = np.eye(
        SLAB, dtype=np.float16)
    wpack[0:R, WCOL_B1] = b1.astype(np.float16)

    w2pack = np.concatenate(
        [ln1_g[:, None] * W2, (b2 + ln1_b @ W2)[None, :]],
        axis=0).astype(np.float16)

    shared = {"wpack": wpack, "w2pack": np.ascontiguousarray(w2pack)}
    in_maps = []
    for c in range(N_CORES):
        sel = obs[BPC * c:BPC * (c + 1)][:, IDX, :]        # [BPC, 428, 512]
        m = {}
        for h in range(BPC):
            obsT = sel[h].T                                 # [512, 428]
            obsf = obsT.reshape(DC, 128, GRP).transpose(1, 0, 2)
            m[f"obs{h}"] = np.ascontiguousarray(obsf.astype(np.float16))
        in_maps.append({**m, **shared})
    return in_maps


def _install_ntff_shim():
    """The agent image's antenv lacks axon_hooks; synthesize it so
    trace=True can reach the libaxon NTFF profiler (test-time only)."""
    import sys
    import types
    if "antenv.axon_hooks" in sys.modules:
        return True
    try:
        import antenv
        from trn_agent_boot.trn_boot import _ntff_profile_via_ctypes
    except ImportError:
        return False
    so_path = "/opt/axon/libaxon_pjrt.so"
    if not os.path.exists(so_path):
        return False
    hook = _ntff_profile_via_ctypes(so_path)
    mod = types.ModuleType("antenv.axon_hooks")
    mod._hook = hook
    mod.set_axon_ntff_profile_hook = lambda h: setattr(mod, "_hook", h)
    mod.get_axon_ntff_profile_hook = lambda: mod._hook
    sys.modules["antenv.axon_hooks"] = mod
    antenv.axon_hooks = mod
    return hook is not None


def kernel(obs_frames, W1, b1, ln1_g, ln1_b, W2, b2, ln2_g, ln2_b):
    nc = _build_program()
    in_maps = _host_inputs(obs_frames, W1, b1, ln1_g, ln1_b, W2, b2)
    trace = bool(os.environ.get("BASS_TRACE"))
    if trace:
        trace = _install_ntff_shim()
        import concourse.bass_utils as _bu
        _bu.upload_artifacts = lambda tmpdir: f"local://{tmpdir}"
    res = run_bass_kernel_spmd(nc, in_maps, core_ids=list(range(N_CORES)),
                               trace=trace)
    _cache["last_result"] = res
    out = np.stack([res.results[c]["out"].astype(np.float32)
                    .reshape(BPC, LOCAL, D) for c in range(N_CORES)])
    out = out.reshape(B, LOCAL, D)

    # LN2's affine applied host-side (identity for the given inputs)
    g2 = np.asarray(ln2_g, np.float32)
    b2l = np.asarray(ln2_b, np.float32)
    if not (np.all(g2 == 1.0) and np.all(b2l == 0.0)):
        out = out * g2 + b2l
    return np.ascontiguousarray(out.astype(np.float32))
